# revision 6
# baseline (speedup 1.0000x reference)
"""Trainium2 Bass kernel for nn_Decoder_31696858644737.

Cosine-similarity attention decoder:
  et = swap(encoder_time)  [B, N, S, 4]
  dt = swap(decoder_time)  [B, N, P, 4]
  scores = cos_sim(dt, et) [B, N, P, S]
  attn = softmax(scores, -1)
  out  = attn @ encoder_target  [B, N, P, D]
returns (attn, out).

Sharding: data-parallel over B (16 batches / 8 cores = 2 per core).

Per-core algorithm (B'=2, N=256 pairs each; P=S=96, D=256):
  Phase 1: normalize time tensors in natural layout [96, N, 4] (per-partition
  rsqrt scaling), PE-transpose to [(n d), p] and round-trip through DRAM
  scratch laid out [n, d, p] so phase 2 can load per-pair [4, 96] tiles at
  SBUF base partition 0 with >=384B-run DMAs.
  Phase 2 per pair: two K=4 matmuls give both score orientations ([p,s] and
  [s,p]); exp on ACT (accum_out gives the softmax row-sum for free);
  attn = E_ps * rinv (per-partition scalar); out = (E_sp.T @ tgt) * rinv.
  Softmax skips max-subtraction: cosine scores are in [-1, 1].
"""

import numpy as np
from contextlib import ExitStack

import concourse.bass as bass
import concourse.bacc as bacc
import concourse.mybir as mybir
import concourse.tile as tile
from concourse.masks import make_identity

F32 = mybir.dt.float32
AF = mybir.ActivationFunctionType
AX = mybir.AxisListType

N_CORES = 8
B_FULL = 16
BPC = B_FULL // N_CORES  # batches per core
NN = 256                 # nodes (pure batch dim)
P = 96                   # decoder len
S = 96                   # encoder len
TD = 4                   # time feature dim
D = 256                  # target dim
G = 8                    # pairs per group (DMA batch)


def build(bpc=BPC, nn=NN, g=G):
    nc = bacc.Bacc(None, target_bir_lowering=False, debug=False)

    dec = nc.dram_tensor("decoder_time", [bpc, P, nn, TD], F32, kind="ExternalInput")
    enc = nc.dram_tensor("encoder_time", [bpc, S, nn, TD], F32, kind="ExternalInput")
    tgt = nc.dram_tensor("encoder_target", [bpc, nn, S, D], F32, kind="ExternalInput")
    attn_o = nc.dram_tensor("attn", [bpc, nn, P, S], F32, kind="ExternalOutput")
    out_o = nc.dram_tensor("target_output", [bpc, nn, P, D], F32, kind="ExternalOutput")
    # scratch: normalized+transposed time tensors, laid out [n, d, p]
    scr_d = nc.dram_tensor("scratch_d", [bpc, nn, TD, P], F32)
    scr_e = nc.dram_tensor("scratch_e", [bpc, nn, TD, S], F32)

    assert nn % 32 == 0 and nn % g == 0

    with tile.TileContext(nc) as tc:
        # ---------------- Phase 1: normalize + transpose to scratch ---------
        with (
            tc.tile_pool(name="ph1", bufs=2) as ph1,
            tc.tile_pool(name="ph1c", bufs=1) as ph1c,
            tc.tile_pool(name="ph1t", bufs=4) as ph1t,
            tc.tile_pool(name="ph1ps", bufs=4, space="PSUM") as ph1ps,
        ):
            ident = ph1c.tile([P, P], F32)
            make_identity(nc, ident)
            for b in range(bpc):
                for src, scr in ((dec, scr_d), (enc, scr_e)):
                    nat = ph1.tile([P, nn, TD], F32, tag="nat")
                    nc.sync.dma_start(out=nat, in_=src[b])
                    sq = ph1.tile([P, nn, TD], F32, tag="sq")
                    nc.vector.tensor_mul(sq, nat, nat)
                    ns = ph1.tile([P, nn], F32, tag="ns")
                    nc.vector.reduce_sum(ns, sq, axis=AX.X)
                    nrm = ph1.tile([P, nn], F32, tag="nrm")
                    nc.scalar.activation(nrm, ns, AF.Sqrt)
                    rinv = ph1.tile([P, nn], F32, tag="rinv")
                    nc.vector.reciprocal(rinv, nrm)
                    that = ph1.tile([P, nn, TD], F32, tag="that")
                    rinv3 = rinv.rearrange("p (n o) -> p n o", o=1)
                    nc.vector.tensor_mul(that, nat, rinv3.to_broadcast((P, nn, TD)))
                    thatf = that.rearrange("p n d -> p (n d)")
                    for t in range(nn // 32):
                        psT = ph1ps.tile([32 * TD, P], F32, tag="psT")
                        nc.tensor.transpose(
                            psT, thatf[:, t * 128:(t + 1) * 128], ident
                        )
                        sbT = ph1t.tile([32 * TD, P], F32, tag="sbT")
                        nc.vector.tensor_copy(sbT, psT)
                        nc.sync.dma_start(
                            out=scr[b, t * 32:(t + 1) * 32].rearrange(
                                "n d p -> (n d) p"
                            ),
                            in_=sbT,
                        )
        tc.strict_bb_all_engine_barrier()

        # ---------------- Phase 2: per-pair attention ------------------------
        with (
            tc.tile_pool(name="timep", bufs=3) as timep,
            tc.tile_pool(name="big", bufs=2) as big,
            tc.tile_pool(name="sm", bufs=4) as sm,
            tc.tile_pool(name="psA", bufs=2, space="PSUM") as psA,
            tc.tile_pool(name="psB", bufs=2, space="PSUM") as psB,
            tc.tile_pool(name="psC", bufs=2, space="PSUM") as psC,
        ):
            for b in range(bpc):
                for j in range(nn // g):
                    n0 = j * g
                    dtT = timep.tile([TD, g, P], F32, tag="dtT")
                    nc.sync.dma_start(
                        out=dtT,
                        in_=scr_d[b, n0:n0 + g].rearrange("n d p -> d n p"),
                    )
                    etT = timep.tile([TD, g, S], F32, tag="etT")
                    nc.sync.dma_start(
                        out=etT,
                        in_=scr_e[b, n0:n0 + g].rearrange("n d p -> d n p"),
                    )
                    tgt_t = big.tile([S, g, D], F32, tag="tgt")
                    nc.sync.dma_start(
                        out=tgt_t,
                        in_=tgt[b, n0:n0 + g].rearrange("n s d -> s n d"),
                    )
                    attn_t = big.tile([P, g, S], F32, tag="attn")
                    out_t = big.tile([P, g, D], F32, tag="out")
                    for k in range(g):
                        ps_a = psA.tile([P, S], F32, tag="ps_a")
                        nc.tensor.matmul(
                            ps_a, dtT[:, k], etT[:, k], start=True, stop=True
                        )
                        ps_b = psB.tile([S, P], F32, tag="ps_b")
                        nc.tensor.matmul(
                            ps_b, etT[:, k], dtT[:, k], start=True, stop=True
                        )
                        E_ps = sm.tile([P, S], F32, tag="E_ps")
                        rowsum = sm.tile([P, 1], F32, tag="rowsum")
                        nc.scalar.activation(
                            E_ps, ps_a, AF.Exp, accum_out=rowsum
                        )
                        E_sp = sm.tile([S, P], F32, tag="E_sp")
                        nc.scalar.activation(E_sp, ps_b, AF.Exp)
                        rs_inv = sm.tile([P, 1], F32, tag="rs_inv")
                        nc.vector.reciprocal(rs_inv, rowsum)
                        nc.gpsimd.tensor_scalar_mul(attn_t[:, k], E_ps, rs_inv)
                        ps_c = psC.tile([P, D], F32, tag="ps_c")
                        nc.tensor.matmul(
                            ps_c, E_sp, tgt_t[:, k], start=True, stop=True
                        )
                        nc.vector.tensor_scalar_mul(out_t[:, k], ps_c, rs_inv)
                    nc.scalar.dma_start(
                        out=attn_o[b, n0:n0 + g].rearrange("n p s -> p n s"),
                        in_=attn_t,
                    )
                    nc.scalar.dma_start(
                        out=out_o[b, n0:n0 + g].rearrange("n p d -> p n d"),
                        in_=out_t,
                    )
    nc.compile()
    return nc


_NC_CACHE = {}


def _get_nc():
    if "nc" not in _NC_CACHE:
        _NC_CACHE["nc"] = build()
    return _NC_CACHE["nc"]


def _run(inputs, trace=False):
    from concourse.bass_utils import run_bass_kernel_spmd

    nc = _get_nc()
    dec = np.ascontiguousarray(np.asarray(inputs["decoder_time"], dtype=np.float32))
    enc = np.ascontiguousarray(np.asarray(inputs["encoder_time"], dtype=np.float32))
    tgt = np.ascontiguousarray(np.asarray(inputs["encoder_target"], dtype=np.float32))
    in_maps = []
    for c in range(N_CORES):
        b0 = c * BPC
        in_maps.append(
            {
                "decoder_time": dec[b0:b0 + BPC],
                "encoder_time": enc[b0:b0 + BPC],
                "encoder_target": tgt[b0:b0 + BPC],
            }
        )
    res = run_bass_kernel_spmd(nc, in_maps, list(range(N_CORES)), trace=trace)
    attn = np.concatenate([res.results[c]["attn"] for c in range(N_CORES)], axis=0)
    out = np.concatenate(
        [res.results[c]["target_output"] for c in range(N_CORES)], axis=0
    )
    return (attn, out), res


def kernel(**inputs):
    (attn, out), _ = _run(inputs, trace=False)
    return attn, out


def bench_ns(inputs, iters=10):
    """Wall-clock the sharded NEFF execution with device-resident inputs.

    Donated zero-output buffers are re-uploaded outside the timed window each
    iteration. Returns (min_ns, all_ns). Includes axon dispatch overhead, so
    treat as an upper bound on HW exec time.
    """
    import time
    import jax
    import jax.numpy as jnp
    from jax.sharding import Mesh, PartitionSpec, NamedSharding
    from jax.experimental.shard_map import shard_map
    from concourse import bass2jax, mybir as mb

    nc = _get_nc()
    bass2jax.install_neuronx_cc_hook()

    in_names, out_names, out_avals, zero_shapes = [], [], [], []
    for alloc in nc.m.functions[0].allocations:
        if not isinstance(alloc, mb.MemoryLocationSet):
            continue
        name = alloc.memorylocations[0].name
        if alloc.kind == "ExternalInput":
            if nc.partition_id_tensor is not None and name == nc.partition_id_tensor.name:
                continue
            in_names.append(name)
        elif alloc.kind == "ExternalOutput":
            out_names.append(name)
            out_avals.append(
                jax.core.ShapedArray(tuple(alloc.tensor_shape), mb.dt.np(alloc.dtype))
            )
    n_params = len(in_names)
    all_in_names = in_names + out_names
    if nc.partition_id_tensor is not None:
        all_in_names = all_in_names + [nc.partition_id_tensor.name]

    def _body(*args):
        operands = list(args)
        if nc.partition_id_tensor is not None:
            operands.append(bass2jax.partition_id_tensor())
        outs = bass2jax._bass_exec_p.bind(
            *operands,
            out_avals=tuple(out_avals),
            in_names=tuple(all_in_names),
            out_names=tuple(out_names),
            lowering_input_output_aliases=(),
            sim_require_finite=True,
            sim_require_nnan=True,
            nc=nc,
        )
        return tuple(outs)

    devices = jax.devices()[:N_CORES]
    mesh = Mesh(np.asarray(devices), ("core",))
    donate = tuple(range(n_params, n_params + len(out_names)))
    fn = jax.jit(
        shard_map(
            _body,
            mesh=mesh,
            in_specs=(PartitionSpec("core"),) * (n_params + len(out_names)),
            out_specs=(PartitionSpec("core"),) * len(out_names),
            check_rep=False,
        ),
        donate_argnums=donate,
        keep_unused=True,
    )
    sh = NamedSharding(mesh, PartitionSpec("core"))
    dec = np.asarray(inputs["decoder_time"], dtype=np.float32)
    enc = np.asarray(inputs["encoder_time"], dtype=np.float32)
    tgt = np.asarray(inputs["encoder_target"], dtype=np.float32)
    full = {"decoder_time": dec, "encoder_time": enc, "encoder_target": tgt}
    xin = [jax.device_put(full[name], sh) for name in in_names]
    jax.block_until_ready(xin)

    zero_shapes = [(N_CORES * a.shape[0], *a.shape[1:]) for a in out_avals]
    zero_dtypes = [a.dtype for a in out_avals]
    make_zeros = jax.jit(
        lambda: tuple(
            jnp.zeros(s, d) for s, d in zip(zero_shapes, zero_dtypes)
        ),
        out_shardings=tuple(sh for _ in out_avals),
    )

    times = []
    for it in range(iters + 1):
        zin = make_zeros()
        jax.block_until_ready(zin)
        t0 = time.perf_counter()
        out = fn(*xin, *zin)
        jax.block_until_ready(out)
        t1 = time.perf_counter()
        if it > 0:  # first call may include compile
            times.append((t1 - t0) * 1e9)
        del out
    return min(times), times
